# revision 7
# baseline (speedup 1.0000x reference)
"""Distributed multi-head attention kernel for 8 TRN2 NeuronCores.

Problem: B=2, N=2048, C=1024, H=16 heads, D=64.
  out = softmax((q@Wq)(k@Wk)^T / sqrt(D)) @ (v@Wv) @ Wo   (per head, biases zero)

Sharding: batch x head-group.  Core c owns batch b=c//4 and head group
g=c%4 -> heads [4g, 4g+4) = channel block [256g, 256g+256).
Zero-redundancy: each core projects only its own 256 Q/K/V channels for
its batch, runs attention for its 4 heads over all 2048 queries/keys,
and computes the row-sharded out-proj partial out^T = Wo_s^T @ A^T
(bf16).  The host sums the 4 partials per batch (the "all-reduce" of
the sharding hint, done at gather time) -- no device collectives.

Per-core dataflow (PE inputs bf16, PSUM f32, softmax exp on ScalarE):
  1. Q^T = Wq_s^T @ xq^T -> qT_sb [128 (=2 heads x 64 d), plane, 2048];
     K^T likewise.  ScalarE (16.8M exps = the ~133us bottleneck at
     128 lanes/1.2GHz) must start early and never stall: plane 0 of
     Q^T/K^T is emitted first, head-0 scores start right after, and
     the remaining projection work rides in head-0's score stream.
  2. V' = xv @ Wv_s -> v1_sb [128 keys, kc, h, 65], ones in col 64
     (the softmax denominator accumulates in PV output col 64).
  3. scores: S^T chunk [128 keys, 512 q] = matmul(K^T slice, Q^T slice)
     two chunks per PSUM group; one exp instr [128,1024] (scale=1/8
     folded in; no max-subtraction needed for ~N(0,1) scores) -> P.
  4. PV in the fast orientation: po[128 q, 65] += P_chunk(stationary)
     @ V'_chunk -- full 128-partition utilization, 65-col stream.
     Normalize via per-partition reciprocal of col 64 -> A [q, d].
  5. A -> A^T with DMA XBAR transposes (off the compute engines);
     out^T partial = Wo_s^T @ A^T -> bf16 -> DRAM, pipelined per
     512-query block.
"""

import sys

sys.path.insert(0, "/opt/trn_rl_repo")

from contextlib import ExitStack

import numpy as np
import ml_dtypes

import concourse.bass as bass
import concourse.bacc as bacc
import concourse.mybir as mybir
import concourse.tile as tile
from concourse.bass_utils import run_bass_kernel_spmd

BF16 = mybir.dt.bfloat16
F32 = mybir.dt.float32
Exp = mybir.ActivationFunctionType.Exp
Mult = mybir.AluOpType.mult

B, N, C = 2, 2048, 1024
H, D = 16, 64
HC = 4              # heads per core
CB = HC * D         # own channel block = 256
DV = D + 1          # V cols per head incl. ones column
NCHUNK = N // 128   # 16 key chunks
NQB = N // 512      # 4 query blocks
SCALE = 1.0 / np.sqrt(D)

_CACHE = {}


def build_nc():
    nc = bacc.Bacc("TRN2", target_bir_lowering=False, debug=False, num_devices=8)

    xqT = nc.declare_dram_parameter("xqT", [C, N], BF16, isOutput=False)
    xkT = nc.declare_dram_parameter("xkT", [C, N], BF16, isOutput=False)
    xvT = nc.declare_dram_parameter("xvT", [C, N], BF16, isOutput=False)
    wq = nc.declare_dram_parameter("wq", [C, CB], BF16, isOutput=False)
    wk = nc.declare_dram_parameter("wk", [C, CB], BF16, isOutput=False)
    wv = nc.declare_dram_parameter("wv", [C, CB], BF16, isOutput=False)
    wo = nc.declare_dram_parameter("wo", [CB, C], BF16, isOutput=False)
    outT = nc.declare_dram_parameter("outT", [C, N], BF16, isOutput=True)

    with tile.TileContext(nc) as tc, ExitStack() as top:
        # ---------------- resident SBUF ----------------
        res = top.enter_context(tc.tile_pool(name="res", bufs=1))
        # Q^T / K^T: plane p holds head 2p in rows 0:64, head 2p+1 in 64:128
        qT_sb = res.tile([128, 2 * N], BF16, tag="qT")
        kT_sb = res.tile([128, 2 * N], BF16, tag="kT")
        v1_sb = res.tile([128, NCHUNK * HC * DV], BF16, tag="v1")
        aT0_sb = res.tile([128, N], BF16, tag="aT0")   # A^T rows 0:128 (h 0,1)
        aT1_sb = res.tile([128, N], BF16, tag="aT1")   # A^T rows 128:256 (h 2,3)
        dinv_sb = res.tile([128, HC * NQB * 4], F32, tag="dinv")

        def q_slice(h, qb):
            base = N * (h // 2)
            return qT_sb[64 * (h % 2):64 * (h % 2) + 64,
                         base + 512 * qb:base + 512 * (qb + 1)]

        def k_slice(h, kc):
            base = N * (h // 2)
            return kT_sb[64 * (h % 2):64 * (h % 2) + 64,
                         base + 128 * kc:base + 128 * (kc + 1)]

        v3 = v1_sb[:].rearrange("p (kc h x) -> p kc h x", kc=NCHUNK, x=DV)

        attn_stack = ExitStack()
        spool = attn_stack.enter_context(
            tc.tile_pool(name="spool", bufs=2, space="PSUM"))   # 2x2 banks
        P_pool = attn_stack.enter_context(
            tc.tile_pool(name="P_pool", bufs=40))               # [128,1024] bf16

        P_tiles = {}    # (h, qb, pair) -> tile

        def scores_pair(h, qb, pair):
            """S^T + exp for chunks (2*pair, 2*pair+1) of head h, qblock qb."""
            st = spool.tile([128, 1024], F32, tag="st", name=f"st_{h}_{qb}_{pair}")
            Pp = P_pool.tile([128, 1024], BF16, tag="P", name=f"P_{h}_{qb}_{pair}")
            for i in range(2):
                kc = 2 * pair + i
                nc.tensor.matmul(st[:, 512 * i:512 * (i + 1)],
                                 k_slice(h, kc), q_slice(h, qb),
                                 start=True, stop=True)
            nc.scalar.activation(Pp[:], st[:], Exp, scale=float(SCALE))
            P_tiles[(h, qb, pair)] = Pp

        # ================= projections =================
        wstack = ExitStack()
        wpool = wstack.enter_context(tc.tile_pool(name="wpool", bufs=24))
        xqk_stack = ExitStack()
        xqpool = xqk_stack.enter_context(tc.tile_pool(name="xqpool", bufs=8))
        xkpool = xqk_stack.enter_context(tc.tile_pool(name="xkpool", bufs=8))
        pj_stack = ExitStack()
        qkpool = pj_stack.enter_context(
            tc.tile_pool(name="qkpool", bufs=2, space="PSUM"))  # 2x1 banks
        vppool = pj_stack.enter_context(
            tc.tile_pool(name="vppool", bufs=2, space="PSUM"))  # 2x1 banks

        wq_t, wk_t, wv_t = [], [], []
        xq_t, xk_t = [], []
        for cc in range(8):
            wq_t.append(wpool.tile([128, CB], BF16, tag="w", name=f"wq_t{cc}"))
            nc.sync.dma_start(out=wq_t[cc][:], in_=wq[128 * cc:128 * (cc + 1), :])
            xq_t.append(xqpool.tile([128, N], BF16, tag="xq", name=f"xq_t{cc}"))
            nc.sync.dma_start(out=xq_t[cc][:], in_=xqT[128 * cc:128 * (cc + 1), :])
        for cc in range(8):
            wk_t.append(wpool.tile([128, CB], BF16, tag="w", name=f"wk_t{cc}"))
            nc.sync.dma_start(out=wk_t[cc][:], in_=wk[128 * cc:128 * (cc + 1), :])
            xk_t.append(xkpool.tile([128, N], BF16, tag="xk", name=f"xk_t{cc}"))
            nc.sync.dma_start(out=xk_t[cc][:], in_=xkT[128 * cc:128 * (cc + 1), :])
        for cc in range(8):
            wv_t.append(wpool.tile([128, CB], BF16, tag="w", name=f"wv_t{cc}"))
            nc.sync.dma_start(out=wv_t[cc][:], in_=wv[128 * cc:128 * (cc + 1), :])

        def qk_proj_block(w_t, x_t, dst_sb, mb, qb):
            """One [128,512] psum group of the Q^T/K^T projection."""
            ps = qkpool.tile([128, 512], F32, tag="ps",
                             name=f"qk_{id(dst_sb) % 97}_{mb}_{qb}")
            for cc in range(8):
                nc.tensor.matmul(ps[:],
                                 w_t[cc][:, 128 * mb:128 * (mb + 1)],
                                 x_t[cc][:, 512 * qb:512 * (qb + 1)],
                                 start=(cc == 0), stop=(cc == 7))
            nc.vector.tensor_copy(
                dst_sb[:, N * mb + 512 * qb:N * mb + 512 * (qb + 1)], ps[:])

        def v_proj_block(xv_t, tb):
            """V' for key-chunk tb: out[128 keys, 256] -> v1 with ones col."""
            ps = vppool.tile([128, 512], F32, tag="vps", name=f"vps{tb}")
            for cc in range(8):
                nc.tensor.matmul(ps[:, 0:CB],
                                 xv_t[cc][:, 128 * tb:128 * (tb + 1)],
                                 wv_t[cc][:],
                                 start=(cc == 0), stop=(cc == 7))
            nc.vector.tensor_copy(
                v3[:, tb, :, 0:D],
                ps[:, 0:CB].rearrange("p (h d) -> p h d", d=D))

        nc.vector.memset(v3[:, :, :, D:DV], 1.0)

        # plane 0 (heads 0,1) of Q^T then K^T
        for qb in range(NQB):
            qk_proj_block(wq_t, xq_t, qT_sb, 0, qb)
        for qb in range(NQB):
            qk_proj_block(wk_t, xk_t, kT_sb, 0, qb)

        # head-0 scores with the remaining projections interleaved:
        # groups 0-7 carry plane 1 of Q^T/K^T, then xq/xk free and the
        # xv DMA starts; groups 8-23 carry the 16 V' blocks.
        for gi in range(8):
            scores_pair(0, gi // 8, gi % 8)
            if gi < 4:
                qk_proj_block(wq_t, xq_t, qT_sb, 1, gi)
            else:
                qk_proj_block(wk_t, xk_t, kT_sb, 1, gi - 4)
        xqk_stack.close()

        xv_stack = ExitStack()
        xvpool = xv_stack.enter_context(tc.tile_pool(name="xvpool", bufs=8))
        xv_t = []
        for cc in range(8):
            xv_t.append(xvpool.tile([128, N], BF16, tag="xv", name=f"xv_t{cc}"))
            nc.sync.dma_start(out=xv_t[cc][:], in_=xvT[128 * cc:128 * (cc + 1), :])

        for gi in range(8, 32):
            scores_pair(0, gi // 8, gi % 8)
            if gi - 8 < NCHUNK:
                v_proj_block(xv_t, gi - 8)

        pj_stack.close()
        xv_stack.close()
        wstack.close()

        # ================= attention + output =================
        opool = attn_stack.enter_context(
            tc.tile_pool(name="opool", bufs=2, space="PSUM"))   # PV psums
        oppool = attn_stack.enter_context(
            tc.tile_pool(name="oppool", bufs=2, space="PSUM"))  # out-proj psums
        apool = attn_stack.enter_context(tc.tile_pool(name="apool", bufs=4))
        ospool = attn_stack.enter_context(tc.tile_pool(name="ospool", bufs=3))
        wopool = attn_stack.enter_context(tc.tile_pool(name="wopool", bufs=2))

        wo_t = [wopool.tile([128, C], BF16, tag="wo", name=f"wo_t{j}")
                for j in range(2)]
        for j in range(2):
            nc.sync.dma_start(out=wo_t[j][:], in_=wo[128 * j:128 * (j + 1), :])

        A_tiles = {}

        def pv_block(h, qb):
            """PV + normalize for head h, query block qb (4 sub-blocks)."""
            if qb not in A_tiles:
                A_tiles[qb] = apool.tile([128, 4 * CB], BF16, tag="A",
                                         name=f"A_{qb}")
            A = A_tiles[qb]
            po = opool.tile([128, 512], F32, tag="po", name=f"po{h}_{qb}")
            for sub in range(4):
                for kc in range(NCHUNK):
                    Pp = P_tiles[(h, qb, kc // 2)]
                    nc.tensor.matmul(
                        po[:, DV * sub:DV * (sub + 1)],
                        Pp[:, 512 * (kc % 2) + 128 * sub:
                           512 * (kc % 2) + 128 * (sub + 1)],
                        v3[:, kc, h, :],
                        start=(kc == 0), stop=(kc == NCHUNK - 1))
            # per-partition denominators -> reciprocal -> normalize
            dcol = dinv_sb[:, 16 * h + 4 * qb:16 * h + 4 * qb + 4]
            nc.vector.reciprocal_approx_fast(
                dcol, po[:, 0:4 * DV].rearrange("p (s v) -> p s v", v=DV)[:, :, D])
            for sub in range(4):
                nc.vector.tensor_scalar(
                    A[:, CB * sub + D * h:CB * sub + D * (h + 1)],
                    po[:, DV * sub:DV * sub + D],
                    dcol[:, sub:sub + 1], None, Mult)
            for pair in range(8):
                del P_tiles[(h, qb, pair)]

        def transposes(qb, db):
            """A[qb] sub-blocks x head-pair db -> aT via DMA XBAR."""
            dst = aT0_sb if db == 0 else aT1_sb
            for sub in range(4):
                nc.sync.dma_start_transpose(
                    out=dst[:, 512 * qb + 128 * sub:512 * qb + 128 * (sub + 1)],
                    in_=A_tiles[qb][:, CB * sub + 128 * db:CB * sub + 128 * (db + 1)])

        def out_proj(qb):
            for m in range(8):
                ps = oppool.tile([128, 512], F32, tag="ops", name=f"ops{m}_{qb}")
                for j, aT in enumerate((aT0_sb, aT1_sb)):
                    nc.tensor.matmul(ps[:], wo_t[j][:, 128 * m:128 * (m + 1)],
                                     aT[:, 512 * qb:512 * (qb + 1)],
                                     start=(j == 0), stop=(j == 1))
                ev = ospool.tile([128, 512], BF16, tag="ev", name=f"oev{m}_{qb}")
                nc.vector.tensor_copy(ev[:], ps[:])
                nc.sync.dma_start(
                    out=outT[128 * m:128 * (m + 1), 512 * qb:512 * (qb + 1)],
                    in_=ev[:])

        for h in range(1, HC):
            for qb in range(NQB):
                pv_block(h - 1, qb)
                for pair in range(8):
                    scores_pair(h, qb, pair)
                if h == 2:
                    transposes(qb, 0)   # heads 0,1 of A[qb] are final
        for qb in range(NQB):
            pv_block(HC - 1, qb)
            transposes(qb, 1)
            if qb > 0:
                out_proj(qb - 1)
        out_proj(NQB - 1)
        attn_stack.close()

    nc.compile()
    return nc


def _get_nc():
    if "nc" not in _CACHE:
        _CACHE["nc"] = build_nc()
    return _CACHE["nc"]


def _make_in_maps(q, k, v, Wq, Wk, Wv, Wo):
    bf = ml_dtypes.bfloat16
    q, k, v = np.asarray(q), np.asarray(k), np.asarray(v)
    qT = [np.ascontiguousarray(q[b].T).astype(bf) for b in range(B)]
    kT = [np.ascontiguousarray(k[b].T).astype(bf) for b in range(B)]
    vT = [np.ascontiguousarray(v[b].T).astype(bf) for b in range(B)]
    Wq, Wk, Wv, Wo = (np.asarray(x) for x in (Wq, Wk, Wv, Wo))
    wq_s = [np.ascontiguousarray(Wq[:, CB * g:CB * (g + 1)]).astype(bf)
            for g in range(4)]
    wk_s = [np.ascontiguousarray(Wk[:, CB * g:CB * (g + 1)]).astype(bf)
            for g in range(4)]
    wv_s = [np.ascontiguousarray(Wv[:, CB * g:CB * (g + 1)]).astype(bf)
            for g in range(4)]
    wo_s = [np.ascontiguousarray(Wo[CB * g:CB * (g + 1), :]).astype(bf)
            for g in range(4)]
    in_maps = []
    for c in range(8):
        b, g = c // 4, c % 4
        in_maps.append({
            "xqT": qT[b], "xkT": kT[b], "xvT": vT[b],
            "wq": wq_s[g], "wk": wk_s[g], "wv": wv_s[g], "wo": wo_s[g],
        })
    return in_maps


def _run(inputs, trace=False, **kw):
    nc = _get_nc()
    in_maps = _make_in_maps(inputs["q"], inputs["k"], inputs["v"],
                            inputs["Wq"], inputs["Wk"], inputs["Wv"], inputs["Wo"])
    res = None
    for attempt in range(3):
        try:
            res = run_bass_kernel_spmd(nc, in_maps, core_ids=list(range(8)),
                                       trace=trace, **kw)
            break
        except Exception:
            if attempt == 2:
                raise
            import time
            time.sleep(2.0)
    out = np.empty((B, N, C), np.float32)
    for b in range(B):
        acc = np.zeros((C, N), np.float32)
        for g in range(4):
            acc += res.results[4 * b + g]["outT"].astype(np.float32)
        out[b] = acc.T
    return out, res


def kernel(**inputs) -> np.ndarray:
    out, _ = _run(inputs, trace=False)
    return out


# revision 15
# speedup vs baseline: 1.3772x; 1.3772x over previous
"""Distributed multi-head attention kernel for 8 TRN2 NeuronCores.

Problem: B=2, N=2048, C=1024, H=16 heads, D=64.
  out = softmax((q@Wq)(k@Wk)^T / sqrt(D)) @ (v@Wv) @ Wo   (per head, biases zero)

Sharding: batch x head-group.  Core c owns batch b=c//4 and head group
g=c%4 -> heads [4g, 4g+4) = channel block [256g, 256g+256).
Zero-redundancy: each core projects only its own 256 Q/K/V channels for
its batch, runs attention for its 4 heads over all 2048 queries/keys,
and computes the row-sharded out-proj partial out^T = Wo_s^T @ A^T
(bf16).  The host sums the 4 partials per batch (the "all-reduce" of
the sharding hint, done at gather time) -- no device collectives.

Per-core dataflow (PE inputs bf16, PSUM f32, softmax exp on ScalarE):
  1. Q^T = Wq_s^T @ xq^T -> qT_sb [128 (=2 heads x 64 d), plane, 2048];
     K^T likewise.  ScalarE (16.8M exps = the ~133us bottleneck at
     128 lanes/1.2GHz) must start early and never stall: plane 0 of
     Q^T/K^T is emitted first, head-0 scores start right after, and
     the remaining projection work rides in head-0's score stream.
  2. V' = xv @ Wv_s -> v1_sb [128 keys, kc, h, 65], ones in col 64
     (the softmax denominator accumulates in PV output col 64).
  3. scores: S^T chunk [128 keys, 512 q] = matmul(K^T slice, Q^T slice)
     two chunks per PSUM group; one exp instr [128,1024] (scale=1/8
     folded in; no max-subtraction needed for ~N(0,1) scores) -> P.
  4. PV V'-stationary: po[128, 512 q] += V'(kc,h) @ P^T chunk; row 64
     accumulates the softmax denominator via the ones column.  V' is
     padded to 128 weight cols so ldweights takes the compiler's
     fast-weight-load path and hides under the 512-col streams
     (a P-stationary variant was 1024 serial 128-col weight reloads =
     +214us of PE time -- measured, not theoretical).
     Normalize: reciprocal -> gpsimd partition broadcast -> multiply,
     writing A^T planes directly (no transposes anywhere).
  5. out^T partial = Wo_s^T @ A^T -> bf16 -> DRAM, pipelined per
     512-query block.
"""

import sys

sys.path.insert(0, "/opt/trn_rl_repo")

from contextlib import ExitStack

import numpy as np
import ml_dtypes

import concourse.bass as bass
import concourse.bacc as bacc
import concourse.mybir as mybir
import concourse.tile as tile
from concourse.bass_utils import run_bass_kernel_spmd

BF16 = mybir.dt.bfloat16
F32 = mybir.dt.float32
Exp = mybir.ActivationFunctionType.Exp
Mult = mybir.AluOpType.mult

B, N, C = 2, 2048, 1024
H, D = 16, 64
HC = 4              # heads per core
CB = HC * D         # own channel block = 256
DV = D + 1          # V cols per head incl. ones column
NCHUNK = N // 128   # 16 key chunks
NQB = N // 512      # 4 query blocks
SCALE = 1.0 / np.sqrt(D)

_CACHE = {}


def build_nc():
    nc = bacc.Bacc("TRN2", target_bir_lowering=False, debug=False, num_devices=8)

    xqT = nc.declare_dram_parameter("xqT", [C, N], BF16, isOutput=False)
    xkT = nc.declare_dram_parameter("xkT", [C, N], BF16, isOutput=False)
    xvT = nc.declare_dram_parameter("xvT", [C, N], BF16, isOutput=False)
    wq = nc.declare_dram_parameter("wq", [C, CB], BF16, isOutput=False)
    wk = nc.declare_dram_parameter("wk", [C, CB], BF16, isOutput=False)
    wv = nc.declare_dram_parameter("wv", [C, CB], BF16, isOutput=False)
    wo = nc.declare_dram_parameter("wo", [CB, C], BF16, isOutput=False)
    outT = nc.declare_dram_parameter("outT", [C, N], BF16, isOutput=True)

    with tile.TileContext(nc) as tc, ExitStack() as top:
        # ---------------- resident SBUF ----------------
        res = top.enter_context(tc.tile_pool(name="res", bufs=1))
        # Q^T / K^T: plane p holds head 2p in rows 0:64, head 2p+1 in 64:128
        qT_sb = res.tile([128, 2 * N], BF16, tag="qT")
        kT_sb = res.tile([128, 2 * N], BF16, tag="kT")
        # V' padded to 128 cols per (kc, h) so PV's ldweights hits the
        # compiler's Fast-Weight-Load path (needs exactly 128 weight
        # cols): col 64 = ones (denominator), cols 65+ = zeros.
        v1_sb = res.tile([128, NCHUNK * HC * 128], BF16, tag="v1")
        aT0_sb = res.tile([128, N], BF16, tag="aT0")   # A^T rows 0:128 (h 0,1)
        aT1_sb = res.tile([128, N], BF16, tag="aT1")   # A^T rows 128:256 (h 2,3)

        def q_slice(h, qb):
            base = N * (h // 2)
            return qT_sb[64 * (h % 2):64 * (h % 2) + 64,
                         base + 512 * qb:base + 512 * (qb + 1)]

        def k_slice(h, kc):
            base = N * (h // 2)
            return kT_sb[64 * (h % 2):64 * (h % 2) + 64,
                         base + 128 * kc:base + 128 * (kc + 1)]

        v3 = v1_sb[:].rearrange("p (kc h x) -> p kc h x", kc=NCHUNK, x=128)

        attn_stack = ExitStack()
        spool = attn_stack.enter_context(
            tc.tile_pool(name="spool", bufs=2, space="PSUM"))   # 2x2 banks
        P_pool = attn_stack.enter_context(
            tc.tile_pool(name="P_pool", bufs=40))               # [128,1024] bf16

        P_tiles = {}    # (h, qb, pair) -> tile

        def scores_pair(h, qb, pair):
            """S^T + exp for chunks (2*pair, 2*pair+1) of head h, qblock qb."""
            st = spool.tile([128, 1024], F32, tag="st", name=f"st_{h}_{qb}_{pair}")
            Pp = P_pool.tile([128, 1024], BF16, tag="P", name=f"P_{h}_{qb}_{pair}")
            for i in range(2):
                kc = 2 * pair + i
                nc.tensor.matmul(st[:, 512 * i:512 * (i + 1)],
                                 k_slice(h, kc), q_slice(h, qb),
                                 start=True, stop=True)
            nc.scalar.activation(Pp[:], st[:], Exp, scale=float(SCALE))
            P_tiles[(h, qb, pair)] = Pp

        # ================= projections =================
        wstack = ExitStack()
        wpool = wstack.enter_context(tc.tile_pool(name="wpool", bufs=24))
        xqk_stack = ExitStack()
        xqpool = xqk_stack.enter_context(tc.tile_pool(name="xqpool", bufs=8))
        xkpool = xqk_stack.enter_context(tc.tile_pool(name="xkpool", bufs=8))
        pj_stack = ExitStack()
        qkpool = pj_stack.enter_context(
            tc.tile_pool(name="qkpool", bufs=2, space="PSUM"))  # 2x1 banks
        vppool = pj_stack.enter_context(
            tc.tile_pool(name="vppool", bufs=2, space="PSUM"))  # 2x1 banks

        wq_t, wk_t, wv_t = [], [], []
        xq_t, xk_t = [], []
        for cc in range(8):
            wq_t.append(wpool.tile([128, CB], BF16, tag="w", name=f"wq_t{cc}"))
            nc.sync.dma_start(out=wq_t[cc][:], in_=wq[128 * cc:128 * (cc + 1), :])
            xq_t.append(xqpool.tile([128, N], BF16, tag="xq", name=f"xq_t{cc}"))
            nc.sync.dma_start(out=xq_t[cc][:], in_=xqT[128 * cc:128 * (cc + 1), :])
        for cc in range(8):
            wk_t.append(wpool.tile([128, CB], BF16, tag="w", name=f"wk_t{cc}"))
            nc.sync.dma_start(out=wk_t[cc][:], in_=wk[128 * cc:128 * (cc + 1), :])
            xk_t.append(xkpool.tile([128, N], BF16, tag="xk", name=f"xk_t{cc}"))
            nc.sync.dma_start(out=xk_t[cc][:], in_=xkT[128 * cc:128 * (cc + 1), :])
        for cc in range(8):
            wv_t.append(wpool.tile([128, CB], BF16, tag="w", name=f"wv_t{cc}"))
            nc.sync.dma_start(out=wv_t[cc][:], in_=wv[128 * cc:128 * (cc + 1), :])

        def qk_proj_block(w_t, x_t, dst_sb, mb, qb):
            """One [128,512] psum group of the Q^T/K^T projection."""
            ps = qkpool.tile([128, 512], F32, tag="ps",
                             name=f"qk_{id(dst_sb) % 97}_{mb}_{qb}")
            for cc in range(8):
                nc.tensor.matmul(ps[:],
                                 w_t[cc][:, 128 * mb:128 * (mb + 1)],
                                 x_t[cc][:, 512 * qb:512 * (qb + 1)],
                                 start=(cc == 0), stop=(cc == 7))
            nc.vector.tensor_copy(
                dst_sb[:, N * mb + 512 * qb:N * mb + 512 * (qb + 1)], ps[:])

        def v_proj_block(xv_t, tb):
            """V' for key-chunk tb: out[128 keys, 256] -> v1 with ones col."""
            ps = vppool.tile([128, 512], F32, tag="vps", name=f"vps{tb}")
            for cc in range(8):
                nc.tensor.matmul(ps[:, 0:CB],
                                 xv_t[cc][:, 128 * tb:128 * (tb + 1)],
                                 wv_t[cc][:],
                                 start=(cc == 0), stop=(cc == 7))
            nc.vector.tensor_copy(
                v3[:, tb, :, 0:D],
                ps[:, 0:CB].rearrange("p (h d) -> p h d", d=D))

        nc.vector.memset(v3[:, :, :, D:DV], 1.0)
        nc.vector.memset(v3[:, :, :, DV:128], 0.0)

        # plane 0 (heads 0,1) of Q^T then K^T
        for qb in range(NQB):
            qk_proj_block(wq_t, xq_t, qT_sb, 0, qb)
        for qb in range(NQB):
            qk_proj_block(wk_t, xk_t, kT_sb, 0, qb)

        # head-0 scores with the remaining projections interleaved:
        # groups 0-7 carry plane 1 of Q^T/K^T, then xq/xk free and the
        # xv DMA starts; groups 8-23 carry the 16 V' blocks.
        for gi in range(8):
            scores_pair(0, gi // 8, gi % 8)
            if gi < 4:
                qk_proj_block(wq_t, xq_t, qT_sb, 1, gi)
            else:
                qk_proj_block(wk_t, xk_t, kT_sb, 1, gi - 4)
        xqk_stack.close()

        xv_stack = ExitStack()
        xvpool = xv_stack.enter_context(tc.tile_pool(name="xvpool", bufs=8))
        xv_t = []
        for cc in range(8):
            xv_t.append(xvpool.tile([128, N], BF16, tag="xv", name=f"xv_t{cc}"))
            nc.sync.dma_start(out=xv_t[cc][:], in_=xvT[128 * cc:128 * (cc + 1), :])

        for gi in range(8, 32):
            scores_pair(0, gi // 8, gi % 8)
            if gi - 8 < NCHUNK:
                v_proj_block(xv_t, gi - 8)

        pj_stack.close()
        xv_stack.close()
        wstack.close()

        # ================= attention + output =================
        opool = attn_stack.enter_context(
            tc.tile_pool(name="opool", bufs=2, space="PSUM"))   # PV psums
        oppool = attn_stack.enter_context(
            tc.tile_pool(name="oppool", bufs=2, space="PSUM"))  # out-proj psums
        dpool = attn_stack.enter_context(tc.tile_pool(name="dpool", bufs=2))
        ospool = attn_stack.enter_context(tc.tile_pool(name="ospool", bufs=3))
        wopool = attn_stack.enter_context(tc.tile_pool(name="wopool", bufs=2))

        wo_t = [wopool.tile([128, C], BF16, tag="wo", name=f"wo_t{j}")
                for j in range(2)]
        for j in range(2):
            nc.sync.dma_start(out=wo_t[j][:], in_=wo[128 * j:128 * (j + 1), :])

        def pv_block(h, qb):
            """PV + normalize for head h, query block qb.

            po rows 0:64 = O^T(h) raw, row 64 = softmax denominator;
            normalized A^T lands in aT plane h//2 rows 64*(h%2).
            """
            po = opool.tile([128, 512], F32, tag="po", name=f"po{h}_{qb}")
            for kc in range(NCHUNK):
                Pp = P_tiles[(h, qb, kc // 2)]
                nc.tensor.matmul(po[:],
                                 v3[:, kc, h, :],
                                 Pp[:, 512 * (kc % 2):512 * (kc % 2 + 1)],
                                 start=(kc == 0), stop=(kc == NCHUNK - 1))
            draw = dpool.tile([1, 512], F32, tag="draw", name=f"dw{h}_{qb}")
            drow = dpool.tile([1, 512], F32, tag="drow", name=f"dr{h}_{qb}")
            dinv = dpool.tile([64, 512], F32, tag="dinv", name=f"di{h}_{qb}")
            nc.vector.tensor_copy(draw[:], po[64:65, :])
            nc.vector.reciprocal_approx_fast(drow[:], draw[:])
            nc.gpsimd.partition_broadcast(dinv[:], drow[:])
            dst = aT0_sb if h < 2 else aT1_sb
            nc.vector.tensor_mul(
                dst[64 * (h % 2):64 * (h % 2) + 64, 512 * qb:512 * (qb + 1)],
                po[0:D, :], dinv[:])
            for pair in range(8):
                del P_tiles[(h, qb, pair)]

        def out_proj(qb):
            for m in range(8):
                ps = oppool.tile([128, 512], F32, tag="ops", name=f"ops{m}_{qb}")
                for j, aT in enumerate((aT0_sb, aT1_sb)):
                    nc.tensor.matmul(ps[:], wo_t[j][:, 128 * m:128 * (m + 1)],
                                     aT[:, 512 * qb:512 * (qb + 1)],
                                     start=(j == 0), stop=(j == 1))
                ev = ospool.tile([128, 512], BF16, tag="ev", name=f"oev{m}_{qb}")
                nc.vector.tensor_copy(ev[:], ps[:])
                nc.sync.dma_start(
                    out=outT[128 * m:128 * (m + 1), 512 * qb:512 * (qb + 1)],
                    in_=ev[:])

        for h in range(1, HC):
            for qb in range(NQB):
                pv_block(h - 1, qb)
                for pair in range(8):
                    scores_pair(h, qb, pair)
        for qb in range(NQB):
            pv_block(HC - 1, qb)
            if qb > 0:
                out_proj(qb - 1)
        out_proj(NQB - 1)
        attn_stack.close()

    nc.compile()
    return nc


def _get_nc():
    if "nc" not in _CACHE:
        _CACHE["nc"] = build_nc()
    return _CACHE["nc"]


def _make_in_maps(q, k, v, Wq, Wk, Wv, Wo):
    bf = ml_dtypes.bfloat16
    q, k, v = np.asarray(q), np.asarray(k), np.asarray(v)
    qT = [np.ascontiguousarray(q[b].T).astype(bf) for b in range(B)]
    kT = [np.ascontiguousarray(k[b].T).astype(bf) for b in range(B)]
    vT = [np.ascontiguousarray(v[b].T).astype(bf) for b in range(B)]
    Wq, Wk, Wv, Wo = (np.asarray(x) for x in (Wq, Wk, Wv, Wo))
    wq_s = [np.ascontiguousarray(Wq[:, CB * g:CB * (g + 1)]).astype(bf)
            for g in range(4)]
    wk_s = [np.ascontiguousarray(Wk[:, CB * g:CB * (g + 1)]).astype(bf)
            for g in range(4)]
    wv_s = [np.ascontiguousarray(Wv[:, CB * g:CB * (g + 1)]).astype(bf)
            for g in range(4)]
    wo_s = [np.ascontiguousarray(Wo[CB * g:CB * (g + 1), :]).astype(bf)
            for g in range(4)]
    in_maps = []
    for c in range(8):
        b, g = c // 4, c % 4
        in_maps.append({
            "xqT": qT[b], "xkT": kT[b], "xvT": vT[b],
            "wq": wq_s[g], "wk": wk_s[g], "wv": wv_s[g], "wo": wo_s[g],
        })
    return in_maps


def _run(inputs, trace=False, **kw):
    nc = _get_nc()
    in_maps = _make_in_maps(inputs["q"], inputs["k"], inputs["v"],
                            inputs["Wq"], inputs["Wk"], inputs["Wv"], inputs["Wo"])
    res = None
    for attempt in range(3):
        try:
            res = run_bass_kernel_spmd(nc, in_maps, core_ids=list(range(8)),
                                       trace=trace, **kw)
            break
        except Exception:
            if attempt == 2:
                raise
            import time
            time.sleep(2.0)
    out = np.empty((B, N, C), np.float32)
    for b in range(B):
        acc = np.zeros((C, N), np.float32)
        for g in range(4):
            acc += res.results[4 * b + g]["outT"].astype(np.float32)
        out[b] = acc.T
    return out, res


def kernel(**inputs) -> np.ndarray:
    out, _ = _run(inputs, trace=False)
    return out


# revision 23
# speedup vs baseline: 1.3973x; 1.0145x over previous
"""Distributed multi-head attention kernel for 8 TRN2 NeuronCores.

Problem: B=2, N=2048, C=1024, H=16 heads, D=64.
  out = softmax((q@Wq)(k@Wk)^T / sqrt(D)) @ (v@Wv) @ Wo   (per head, biases zero)

Sharding: batch x head-group.  Core c owns batch b=c//4 and head group
g=c%4 -> heads [4g, 4g+4) = channel block [256g, 256g+256).
Zero-redundancy: each core projects only its own 256 Q/K/V channels for
its batch, runs attention for its 4 heads over all 2048 queries/keys,
and computes the row-sharded out-proj partial out^T = Wo_s^T @ A^T
(bf16).  The host sums the 4 partials per batch (the "all-reduce" of
the sharding hint, done at gather time) -- no device collectives.

The wall-clock floor is ScalarE: 16.8M softmax exps per core at
128 lanes / 1.2 GHz (~1114ns per [128,1024] ACTIVATE = ~143us).
Everything else is scheduled to keep ScalarE saturated from ~24us
(first scores need all of xq+xk on chip) to the end:

  - query-block-major pipeline with one "slot" per score group (one
    exp).  Per 512-query block qb: 4 heads x 8 score pairs stream into
    ScalarE; PV of head h-1 rides pair-by-pair inside head h's slots;
    out-proj of qb-1 and the Q projection of qb+1 ride as fillers.
    After the last exp only one block's PV tail + out-proj remain.
  - PV is V'-stationary (65-col weights; col 64 = ones accumulates the
    softmax denominator).  A P-stationary variant costs +214us in
    serial 128-col weight reloads (measured).
  - normalize: copy denom row -> reciprocal -> gpsimd partition
    broadcast -> multiply, writing A^T planes directly (no transposes).
  - input DMA splits across both hardware DGE queues (SP: xq, wq, wo;
    Activation: xk, wk, wv, xv) so xq and xk stream in parallel.
  - SBUF: x tensors live on the right-side allocator stack and are
    released in LIFO order (xk after the K projection, xv after qb0,
    xq after the last Q projection) so the 44KB P pool fits.
  - PSUM: 4 banks of score groups (2x[128,1024]) + 2 PV banks +
    2 shared banks for qk/v/out-proj groups = exactly 8.
"""

import sys

sys.path.insert(0, "/opt/trn_rl_repo")

from contextlib import ExitStack

import numpy as np
import ml_dtypes

import concourse.bass as bass
import concourse.bacc as bacc
import concourse.mybir as mybir
import concourse.tile as tile
from concourse.bass_utils import run_bass_kernel_spmd

BF16 = mybir.dt.bfloat16
F32 = mybir.dt.float32
Exp = mybir.ActivationFunctionType.Exp

B, N, C = 2, 2048, 1024
H, D = 16, 64
HC = 4              # heads per core
CB = HC * D         # own channel block = 256
DV = D + 1          # V cols per head incl. ones column
NCHUNK = N // 128   # 16 key chunks
NQB = N // 512      # 4 query blocks
SCALE = 1.0 / np.sqrt(D)

_CACHE = {}


def build_nc():
    nc = bacc.Bacc("TRN2", target_bir_lowering=False, debug=False, num_devices=8)

    xqT = nc.declare_dram_parameter("xqT", [C, N], BF16, isOutput=False)
    xkT = nc.declare_dram_parameter("xkT", [C, N], BF16, isOutput=False)
    xvT = nc.declare_dram_parameter("xvT", [C, N], BF16, isOutput=False)
    wq = nc.declare_dram_parameter("wq", [C, CB], BF16, isOutput=False)
    wk = nc.declare_dram_parameter("wk", [C, CB], BF16, isOutput=False)
    wv = nc.declare_dram_parameter("wv", [C, CB], BF16, isOutput=False)
    wo = nc.declare_dram_parameter("wo", [CB, C], BF16, isOutput=False)
    outT = nc.declare_dram_parameter("outT", [C, N], BF16, isOutput=True)

    with tile.TileContext(nc) as tc, ExitStack() as top:
        # ---------------- resident SBUF ----------------
        res = top.enter_context(tc.tile_pool(name="res", bufs=1))
        # Q^T / K^T: plane p holds head 2p in rows 0:64, head 2p+1 in 64:128
        qT_sb = res.tile([128, 2 * N], BF16, tag="qT")
        kT_sb = res.tile([128, 2 * N], BF16, tag="kT")
        v1_sb = res.tile([128, NCHUNK * HC * DV], BF16, tag="v1")
        aT0_sb = res.tile([128, N], BF16, tag="aT0")   # A^T rows 0:128 (h 0,1)
        aT1_sb = res.tile([128, N], BF16, tag="aT1")   # A^T rows 128:256 (h 2,3)

        def q_slice(h, qb):
            base = N * (h // 2)
            return qT_sb[64 * (h % 2):64 * (h % 2) + 64,
                         base + 512 * qb:base + 512 * (qb + 1)]

        def k_slice(h, kc):
            base = N * (h // 2)
            return kT_sb[64 * (h % 2):64 * (h % 2) + 64,
                         base + 128 * kc:base + 128 * (kc + 1)]

        v3 = v1_sb[:].rearrange("p (kc h x) -> p kc h x", kc=NCHUNK, x=DV)

        # ---------------- pools ----------------
        main = ExitStack()
        wpool = main.enter_context(tc.tile_pool(name="wpool", bufs=24))
        wopool = main.enter_context(tc.tile_pool(name="wopool", bufs=2))
        P_pool = main.enter_context(tc.tile_pool(name="P_pool", bufs=22))
        dpool = main.enter_context(tc.tile_pool(name="dpool", bufs=2))
        ospool = main.enter_context(tc.tile_pool(name="ospool", bufs=3))
        spool = main.enter_context(
            tc.tile_pool(name="spool", bufs=2, space="PSUM"))   # 2x2 banks
        pvpool = main.enter_context(
            tc.tile_pool(name="pvpool", bufs=2, space="PSUM"))  # 2x1 banks
        gpool = main.enter_context(
            tc.tile_pool(name="gpool", bufs=2, space="PSUM"))   # 2x1 banks
        xq_stack = ExitStack()
        xqpool = xq_stack.enter_context(
            tc.tile_pool(name="xqpool", bufs=8, side="right"))
        xv_stack = ExitStack()
        xvpool = xv_stack.enter_context(
            tc.tile_pool(name="xvpool", bufs=8, side="right"))
        xk_stack = ExitStack()
        xkpool = xk_stack.enter_context(
            tc.tile_pool(name="xkpool", bufs=8, side="right"))

        # ---------------- input DMA (both DGE queues) ----------------
        wq_t, wk_t, wv_t, xq_t, xk_t, xv_t = [], [], [], [], [], []
        for cc in range(8):
            wq_t.append(wpool.tile([128, CB], BF16, tag="w", name=f"wq_t{cc}"))
            nc.sync.dma_start(out=wq_t[cc][:], in_=wq[128 * cc:128 * (cc + 1), :])
            wk_t.append(wpool.tile([128, CB], BF16, tag="w", name=f"wk_t{cc}"))
            nc.scalar.dma_start(out=wk_t[cc][:], in_=wk[128 * cc:128 * (cc + 1), :])
        for cc in range(8):
            xq_t.append(xqpool.tile([128, N], BF16, tag="xq", name=f"xq_t{cc}"))
            nc.sync.dma_start(out=xq_t[cc][:], in_=xqT[128 * cc:128 * (cc + 1), :])
            xk_t.append(xkpool.tile([128, N], BF16, tag="xk", name=f"xk_t{cc}"))
            nc.scalar.dma_start(out=xk_t[cc][:], in_=xkT[128 * cc:128 * (cc + 1), :])
        for cc in range(8):
            wv_t.append(wpool.tile([128, CB], BF16, tag="w", name=f"wv_t{cc}"))
            nc.scalar.dma_start(out=wv_t[cc][:], in_=wv[128 * cc:128 * (cc + 1), :])
            xv_t.append(xvpool.tile([128, N], BF16, tag="xv", name=f"xv_t{cc}"))
            nc.scalar.dma_start(out=xv_t[cc][:], in_=xvT[128 * cc:128 * (cc + 1), :])
        wo_t = [wopool.tile([128, C], BF16, tag="wo", name=f"wo_t{j}")
                for j in range(2)]
        for j in range(2):
            nc.sync.dma_start(out=wo_t[j][:], in_=wo[128 * j:128 * (j + 1), :])

        nc.vector.memset(v3[:, :, :, D:DV], 1.0)

        # ---------------- building blocks ----------------
        P_tiles, PV, qk_state = {}, {}, {}

        def scores_pair(h, qb, pair):
            """S^T + exp for chunks (2*pair, 2*pair+1) of head h, qblock qb."""
            st = spool.tile([128, 1024], F32, tag="st", name=f"st_{h}_{qb}_{pair}")
            Pp = P_pool.tile([128, 1024], BF16, tag="P", name=f"P_{h}_{qb}_{pair}")
            for i in range(2):
                kc = 2 * pair + i
                nc.tensor.matmul(st[:, 512 * i:512 * (i + 1)],
                                 k_slice(h, kc), q_slice(h, qb),
                                 start=True, stop=True)
            nc.scalar.activation(Pp[:], st[:], Exp, scale=float(SCALE))
            P_tiles[(h, qb, pair)] = Pp

        def qk_proj_half(w_t, x_t, dst_sb, mb, qb, half):
            """Half (4 cc-steps) of one [128,512] Q^T/K^T projection group."""
            key = (id(w_t), mb, qb)
            if half == 0:
                qk_state[key] = gpool.tile([128, 512], F32, tag="g",
                                           name=f"qk{mb}_{qb}_{id(w_t) % 97}")
            ps = qk_state[key]
            for cc in range(4 * half, 4 * half + 4):
                nc.tensor.matmul(ps[:],
                                 w_t[cc][:, 128 * mb:128 * (mb + 1)],
                                 x_t[cc][:, 512 * qb:512 * (qb + 1)],
                                 start=(cc == 0), stop=(cc == 7))
            if half == 1:
                nc.vector.tensor_copy(
                    dst_sb[:, N * mb + 512 * qb:N * mb + 512 * (qb + 1)], ps[:])
                del qk_state[key]

        def v_proj_block(tb):
            """V' for key-chunk tb: out[128 keys, 256] -> v1 cols 0:64."""
            ps = gpool.tile([128, 512], F32, tag="g", name=f"vps{tb}")
            for cc in range(8):
                nc.tensor.matmul(ps[:, 0:CB],
                                 xv_t[cc][:, 128 * tb:128 * (tb + 1)],
                                 wv_t[cc][:],
                                 start=(cc == 0), stop=(cc == 7))
            nc.vector.tensor_copy(
                v3[:, tb, :, 0:D],
                ps[:, 0:CB].rearrange("p (h d) -> p h d", d=D))

        def pv_part(h, qb, pair):
            """Two PV chunk-matmuls for head h / qblock qb; finishes at pair 7.

            po rows 0:64 = O^T(h) raw, row 64 = softmax denominator.
            """
            if pair == 0:
                PV[(h, qb)] = pvpool.tile([128, 512], F32, tag="po",
                                          name=f"po{h}_{qb}")
            po = PV[(h, qb)]
            Pp = P_tiles.pop((h, qb, pair))
            for i in range(2):
                kc = 2 * pair + i
                nc.tensor.matmul(po[0:DV, :],
                                 v3[:, kc, h, :],
                                 Pp[:, 512 * i:512 * (i + 1)],
                                 start=(kc == 0), stop=(kc == NCHUNK - 1))
            if pair == 7:
                pv_finish(h, qb)

        def pv_finish(h, qb):
            """Normalize: A^T(h) = po[0:64] / po[64] -> aT plane."""
            po = PV.pop((h, qb))
            draw = dpool.tile([1, 512], F32, tag="draw", name=f"dw{h}_{qb}")
            drow = dpool.tile([1, 512], F32, tag="drow", name=f"dr{h}_{qb}")
            dinv = dpool.tile([64, 512], F32, tag="dinv", name=f"di{h}_{qb}")
            nc.vector.tensor_copy(draw[:], po[64:65, :])
            nc.vector.reciprocal_approx_fast(drow[:], draw[:])
            nc.gpsimd.partition_broadcast(dinv[:], drow[:])
            dst = aT0_sb if h < 2 else aT1_sb
            nc.vector.tensor_mul(
                dst[64 * (h % 2):64 * (h % 2) + 64, 512 * qb:512 * (qb + 1)],
                po[0:D, :], dinv[:])

        def oproj_m(qb, m):
            """One m-block of the out-proj partial for query block qb."""
            ps = gpool.tile([128, 512], F32, tag="g", name=f"ops{m}_{qb}")
            for j, aT in enumerate((aT0_sb, aT1_sb)):
                nc.tensor.matmul(ps[:], wo_t[j][:, 128 * m:128 * (m + 1)],
                                 aT[:, 512 * qb:512 * (qb + 1)],
                                 start=(j == 0), stop=(j == 1))
            ev = ospool.tile([128, 512], BF16, tag="ev", name=f"oev{m}_{qb}")
            nc.vector.tensor_copy(ev[:], ps[:])
            nc.sync.dma_start(
                out=outT[128 * m:128 * (m + 1), 512 * qb:512 * (qb + 1)],
                in_=ev[:])

        # ---------------- emission ----------------
        # Pre-loop: Q^T(qb0) both planes, K^T plane 0.  First exp fires
        # once these and the xq/xk DMAs land (~24us).
        for mb in range(2):
            for half in range(2):
                qk_proj_half(wq_t, xq_t, qT_sb, mb, 0, half)
        for kb in range(NQB):
            for half in range(2):
                qk_proj_half(wk_t, xk_t, kT_sb, 0, kb, half)

        # Slot schedule.  One slot = one score group = one exp (~1.1us).
        # qb0 (PV lags 2 heads; V' streams just-in-time):
        #   h0 slot p: K^T plane 1 half p      (needed by h2 scores)
        #   h1 slot p: V' chunk 2p
        #   h2 slot p: V' chunk 2p+1 (pre), then pv(h0) pair p
        #   h3 slot p: pv(h1) pair p, +Q^T(qb1) halves on p<4
        #   tail: pv(h2), pv(h3)
        # qb>=1 (PV lags 1 head): fillers = out-proj(qb-1) then Q^T(qb+1),
        # one per slot from the top of the block.
        def run_qblock(qb, pre_items, post_items):
            lag = 2 if qb == 0 else 1
            for h in range(HC):
                for pair in range(8):
                    scores_pair(h, qb, pair)
                    for it in pre_items.get((h, pair), ()):
                        it()
                    if h >= lag:
                        pv_part(h - lag, qb, pair)
                    for it in post_items.get((h, pair), ()):
                        it()
            for h in range(HC - lag, HC):
                for pair in range(8):
                    pv_part(h, qb, pair)

        pre0, post0 = {}, {}
        for p in range(8):
            post0[(0, p)] = [lambda kb=p // 2, half=p % 2:
                             qk_proj_half(wk_t, xk_t, kT_sb, 1, kb, half)]
            post0[(1, p)] = [lambda tb=2 * p: v_proj_block(tb)]
            pre0[(2, p)] = [lambda tb=2 * p + 1: v_proj_block(tb)]
            if p < 4:
                post0[(3, p)] = [lambda mb=p // 2, half=p % 2:
                                 qk_proj_half(wq_t, xq_t, qT_sb, mb, 1, half)]
        run_qblock(0, pre0, post0)
        xk_stack.close()
        xv_stack.close()

        for qb in range(1, NQB):
            items = [lambda m=m, q=qb - 1: oproj_m(q, m) for m in range(8)]
            if qb < NQB - 1:
                items += [lambda mb=mb, half=half, q=qb + 1:
                          qk_proj_half(wq_t, xq_t, qT_sb, mb, q, half)
                          for mb in range(2) for half in range(2)]
            sched = {}
            for s, it in enumerate(items):
                sched[(s // 8, s % 8)] = [it]
            run_qblock(qb, {}, sched)
            if qb == NQB - 2:
                xq_stack.close()
        for m in range(8):
            oproj_m(NQB - 1, m)
        main.close()

    nc.compile()
    return nc


def _get_nc():
    if "nc" not in _CACHE:
        _CACHE["nc"] = build_nc()
    return _CACHE["nc"]


def _make_in_maps(q, k, v, Wq, Wk, Wv, Wo):
    bf = ml_dtypes.bfloat16
    q, k, v = np.asarray(q), np.asarray(k), np.asarray(v)
    qT = [np.ascontiguousarray(q[b].T).astype(bf) for b in range(B)]
    kT = [np.ascontiguousarray(k[b].T).astype(bf) for b in range(B)]
    vT = [np.ascontiguousarray(v[b].T).astype(bf) for b in range(B)]
    Wq, Wk, Wv, Wo = (np.asarray(x) for x in (Wq, Wk, Wv, Wo))
    wq_s = [np.ascontiguousarray(Wq[:, CB * g:CB * (g + 1)]).astype(bf)
            for g in range(4)]
    wk_s = [np.ascontiguousarray(Wk[:, CB * g:CB * (g + 1)]).astype(bf)
            for g in range(4)]
    wv_s = [np.ascontiguousarray(Wv[:, CB * g:CB * (g + 1)]).astype(bf)
            for g in range(4)]
    wo_s = [np.ascontiguousarray(Wo[CB * g:CB * (g + 1), :]).astype(bf)
            for g in range(4)]
    in_maps = []
    for c in range(8):
        b, g = c // 4, c % 4
        in_maps.append({
            "xqT": qT[b], "xkT": kT[b], "xvT": vT[b],
            "wq": wq_s[g], "wk": wk_s[g], "wv": wv_s[g], "wo": wo_s[g],
        })
    return in_maps


def _run(inputs, trace=False, **kw):
    nc = _get_nc()
    in_maps = _make_in_maps(inputs["q"], inputs["k"], inputs["v"],
                            inputs["Wq"], inputs["Wk"], inputs["Wv"], inputs["Wo"])
    res = None
    for attempt in range(3):
        try:
            res = run_bass_kernel_spmd(nc, in_maps, core_ids=list(range(8)),
                                       trace=trace, **kw)
            break
        except Exception:
            if attempt == 2:
                raise
            import time
            time.sleep(2.0)
    out = np.empty((B, N, C), np.float32)
    for b in range(B):
        acc = np.zeros((C, N), np.float32)
        for g in range(4):
            acc += res.results[4 * b + g]["outT"].astype(np.float32)
        out[b] = acc.T
    return out, res


def kernel(**inputs) -> np.ndarray:
    out, _ = _run(inputs, trace=False)
    return out


# revision 24
# speedup vs baseline: 1.4254x; 1.0201x over previous
"""Distributed multi-head attention kernel for 8 TRN2 NeuronCores.

Problem: B=2, N=2048, C=1024, H=16 heads, D=64.
  out = softmax((q@Wq)(k@Wk)^T / sqrt(D)) @ (v@Wv) @ Wo   (per head, biases zero)

Sharding: batch x head-group.  Core c owns batch b=c//4 and head group
g=c%4 -> heads [4g, 4g+4) = channel block [256g, 256g+256).
Zero-redundancy: each core projects only its own 256 Q/K/V channels for
its batch, runs attention for its 4 heads over all 2048 queries/keys,
and computes the row-sharded out-proj partial out^T = Wo_s^T @ A^T
(bf16).  The host sums the 4 partials per batch (the "all-reduce" of
the sharding hint, done at gather time) -- no device collectives.

The wall-clock floor is ScalarE: 16.8M softmax exps per core at
128 lanes / 1.2 GHz (~1114ns per [128,1024] ACTIVATE = ~143us).
Everything else is scheduled to keep ScalarE saturated from as early
as possible to the end:

  - inputs arrive as ONE batched 3D-AP DMA per half-tensor, split
    across both hardware DGE queues (SP: q-side + v-side + wo;
    Activation: k-side), ordered so the ~5MB the first score groups
    need lands in ~20us while the rest streams behind the exps.
  - query-block-major pipeline with one "slot" per score group (one
    exp, ~1.1us).  Per 512-query block qb, 4 heads x 8 score pairs
    stream into ScalarE; PV of lagging heads, out-proj of qb-1, and
    the Q projection of qb+1 ride pair-by-pair inside the slots at
    <=~1.3us PE load per slot.  The last head's PV is slot-lagged by
    one pair so only one pair + normalize + out-proj remain after the
    final exp.
  - PV is V'-stationary, V' padded to 128 weight cols so its ldweights
    takes the fast-weight-load path (65-col loads serialize ~100ns
    each, measured +25us of PE); col 64 = ones accumulates the softmax
    denominator.  A P-stationary PV costs +214us in serial weight
    reloads (measured).
  - normalize: copy denom row -> reciprocal -> gpsimd partition
    broadcast -> multiply, writing A^T planes directly (no transposes).
  - SBUF: x tensors live on the right-side allocator stack, released
    LIFO (xk after the K projection, xv after qb0, xq after the last
    Q projection) so the 42KB P pool + padded V' fit.
  - PSUM: 4 banks of score groups (2x[128,1024]) + 2 PV banks +
    2 shared banks for qk/v/out-proj groups = exactly 8.
"""

import sys

sys.path.insert(0, "/opt/trn_rl_repo")

from contextlib import ExitStack

import numpy as np
import ml_dtypes

import concourse.bass as bass
import concourse.bacc as bacc
import concourse.mybir as mybir
import concourse.tile as tile
from concourse.bass_utils import run_bass_kernel_spmd

BF16 = mybir.dt.bfloat16
F32 = mybir.dt.float32
Exp = mybir.ActivationFunctionType.Exp

B, N, C = 2, 2048, 1024
H, D = 16, 64
HC = 4              # heads per core
CB = HC * D         # own channel block = 256
DV = D + 1          # V cols per head incl. ones column
NCHUNK = N // 128   # 16 key chunks
NQB = N // 512      # 4 query blocks
SCALE = 1.0 / np.sqrt(D)

_CACHE = {}


def build_nc():
    nc = bacc.Bacc("TRN2", target_bir_lowering=False, debug=False, num_devices=8)

    xqT = nc.declare_dram_parameter("xqT", [C, N], BF16, isOutput=False)
    xkT = nc.declare_dram_parameter("xkT", [C, N], BF16, isOutput=False)
    xvT = nc.declare_dram_parameter("xvT", [C, N], BF16, isOutput=False)
    wq = nc.declare_dram_parameter("wq", [C, CB], BF16, isOutput=False)
    wk = nc.declare_dram_parameter("wk", [C, CB], BF16, isOutput=False)
    wv = nc.declare_dram_parameter("wv", [C, CB], BF16, isOutput=False)
    wo = nc.declare_dram_parameter("wo", [CB, C], BF16, isOutput=False)
    outT = nc.declare_dram_parameter("outT", [C, N], BF16, isOutput=True)

    with tile.TileContext(nc) as tc, ExitStack() as top:
        # ---------------- resident SBUF ----------------
        res = top.enter_context(tc.tile_pool(name="res", bufs=1))
        # Q^T / K^T: plane p holds head 2p in rows 0:64, head 2p+1 in 64:128
        qT_sb = res.tile([128, 2 * N], BF16, tag="qT")
        kT_sb = res.tile([128, 2 * N], BF16, tag="kT")
        # V' padded to 128 cols per (kc, h) for the fast-weight-load
        # path: col 64 = ones (denominator), cols 65+ = zeros.
        v1_sb = res.tile([128, NCHUNK * HC * 128], BF16, tag="v1")
        aT0_sb = res.tile([128, N], BF16, tag="aT0")   # A^T rows 0:128 (h 0,1)
        aT1_sb = res.tile([128, N], BF16, tag="aT1")   # A^T rows 128:256 (h 2,3)
        draw_sb = res.tile([1, 512], F32, tag="draw")
        drow_sb = res.tile([1, 512], F32, tag="drow")

        def q_slice(h, qb):
            base = N * (h // 2)
            return qT_sb[64 * (h % 2):64 * (h % 2) + 64,
                         base + 512 * qb:base + 512 * (qb + 1)]

        def k_slice(h, kc):
            base = N * (h // 2)
            return kT_sb[64 * (h % 2):64 * (h % 2) + 64,
                         base + 128 * kc:base + 128 * (kc + 1)]

        v3 = v1_sb[:].rearrange("p (kc h x) -> p kc h x", kc=NCHUNK, x=128)

        # ---------------- pools ----------------
        main = ExitStack()
        wpool = main.enter_context(tc.tile_pool(name="wpool", bufs=4))
        P_pool = main.enter_context(tc.tile_pool(name="P_pool", bufs=21))
        dpool = main.enter_context(tc.tile_pool(name="dpool", bufs=2))
        ospool = main.enter_context(tc.tile_pool(name="ospool", bufs=2))
        spool = main.enter_context(
            tc.tile_pool(name="spool", bufs=2, space="PSUM"))   # 2x2 banks
        pvpool = main.enter_context(
            tc.tile_pool(name="pvpool", bufs=2, space="PSUM"))  # 2x1 banks
        gpool = main.enter_context(
            tc.tile_pool(name="gpool", bufs=2, space="PSUM"))   # 2x1 banks
        xq_stack = ExitStack()
        xqpool = xq_stack.enter_context(
            tc.tile_pool(name="xqpool", bufs=2, side="right"))
        xv_stack = ExitStack()
        xvpool = xv_stack.enter_context(
            tc.tile_pool(name="xvpool", bufs=2, side="right"))
        xk_stack = ExitStack()
        xkpool = xk_stack.enter_context(
            tc.tile_pool(name="xkpool", bufs=2, side="right"))

        # -------- input DMA: one batched transfer per half-tensor --------
        # DRAM [1024, n] viewed as [128 partitions, 8 cc-chunks, n].
        def dram3(t, lo, hi):
            return t[:].rearrange("(c p) n -> p c n", p=128)[:, :, lo:hi]

        wq_t = res.tile([128, 8 * CB], BF16, tag="wqt")
        wk_t = res.tile([128, 8 * CB], BF16, tag="wkt")
        wv_t = res.tile([128, 8 * CB], BF16, tag="wvt")
        wo_t = res.tile([128, 2 * C], BF16, tag="wot")
        wq3 = wq_t[:].rearrange("p (c n) -> p c n", c=8)
        wk3 = wk_t[:].rearrange("p (c n) -> p c n", c=8)
        wv3 = wv_t[:].rearrange("p (c n) -> p c n", c=8)
        wo3 = wo_t[:].rearrange("p (j n) -> p j n", j=2)

        xq_t = [xqpool.tile([128, 8 * 1024], BF16, tag="xq", name=f"xq{i}")
                for i in range(2)]
        xk_t = [xkpool.tile([128, 8 * 1024], BF16, tag="xk", name=f"xk{i}")
                for i in range(2)]
        xv_t = [xvpool.tile([128, 8 * 1024], BF16, tag="xv", name=f"xv{i}")
                for i in range(2)]
        xq3 = [t[:].rearrange("p (c n) -> p c n", c=8) for t in xq_t]
        xk3 = [t[:].rearrange("p (c n) -> p c n", c=8) for t in xk_t]
        xv3 = [t[:].rearrange("p (c n) -> p c n", c=8) for t in xv_t]

        nc.sync.dma_start(out=wq3[:], in_=dram3(wq, 0, CB))
        nc.scalar.dma_start(out=wk3[:], in_=dram3(wk, 0, CB))
        for i in range(2):
            nc.sync.dma_start(out=xq3[i][:], in_=dram3(xqT, 1024 * i, 1024 * (i + 1)))
            nc.scalar.dma_start(out=xk3[i][:], in_=dram3(xkT, 1024 * i, 1024 * (i + 1)))
        nc.sync.dma_start(out=wv3[:], in_=dram3(wv, 0, CB))
        for i in range(2):
            nc.sync.dma_start(out=xv3[i][:], in_=dram3(xvT, 1024 * i, 1024 * (i + 1)))
        nc.sync.dma_start(out=wo3[:],
                          in_=wo[:].rearrange("(j p) n -> p j n", p=128))

        nc.vector.memset(v3[:, :, :, D:DV], 1.0)
        nc.vector.memset(v3[:, :, :, DV:128], 0.0)

        # ---------------- building blocks ----------------
        P_tiles, PV, qk_state = {}, {}, {}

        def scores_pair(h, qb, pair):
            """S^T + exp for chunks (2*pair, 2*pair+1) of head h, qblock qb."""
            st = spool.tile([128, 1024], F32, tag="st", name=f"st_{h}_{qb}_{pair}")
            Pp = P_pool.tile([128, 1024], BF16, tag="P", name=f"P_{h}_{qb}_{pair}")
            for i in range(2):
                kc = 2 * pair + i
                nc.tensor.matmul(st[:, 512 * i:512 * (i + 1)],
                                 k_slice(h, kc), q_slice(h, qb),
                                 start=True, stop=True)
            nc.scalar.activation(Pp[:], st[:], Exp, scale=float(SCALE))
            P_tiles[(h, qb, pair)] = Pp

        def qk_proj_part(w3, x3, dst_sb, mb, qb, part, nparts):
            """1/nparts of one [128,512] Q^T/K^T projection group."""
            key = (id(w3), mb, qb)
            if part == 0:
                qk_state[key] = gpool.tile([128, 512], F32, tag="g",
                                           name=f"qk{mb}_{qb}_{id(w3) % 97}")
            ps = qk_state[key]
            step = 8 // nparts
            for cc in range(step * part, step * (part + 1)):
                nc.tensor.matmul(ps[:],
                                 w3[:, cc, 128 * mb:128 * (mb + 1)],
                                 x3[qb // 2][:, cc, 512 * (qb % 2):
                                             512 * (qb % 2) + 512],
                                 start=(cc == 0), stop=(cc == 7))
            if part == nparts - 1:
                nc.vector.tensor_copy(
                    dst_sb[:, N * mb + 512 * qb:N * mb + 512 * (qb + 1)], ps[:])
                del qk_state[key]

        def v_proj_block(tb):
            """V' for key-chunk tb: out[128 keys, 256] -> v1 cols 0:64."""
            ps = gpool.tile([128, 512], F32, tag="g", name=f"vps{tb}")
            for cc in range(8):
                nc.tensor.matmul(ps[:, 0:CB],
                                 xv3[tb // 8][:, cc, 128 * (tb % 8):
                                              128 * (tb % 8) + 128],
                                 wv3[:, cc, :],
                                 start=(cc == 0), stop=(cc == 7))
            nc.vector.tensor_copy(
                v3[:, tb, :, 0:D],
                ps[:, 0:CB].rearrange("p (h d) -> p h d", d=D))

        def pv_part(h, qb, pair):
            """Two PV chunk-matmuls for head h / qblock qb; finishes at pair 7.

            po rows 0:64 = O^T(h) raw, row 64 = softmax denominator.
            """
            if pair == 0:
                PV[(h, qb)] = pvpool.tile([128, 512], F32, tag="po",
                                          name=f"po{h}_{qb}")
            po = PV[(h, qb)]
            Pp = P_tiles.pop((h, qb, pair))
            for i in range(2):
                kc = 2 * pair + i
                nc.tensor.matmul(po[:],
                                 v3[:, kc, h, :],
                                 Pp[:, 512 * i:512 * (i + 1)],
                                 start=(kc == 0), stop=(kc == NCHUNK - 1))
            if pair == 7:
                pv_finish(h, qb)

        def pv_finish(h, qb):
            """Normalize: A^T(h) = po[0:64] / po[64] -> aT plane."""
            po = PV.pop((h, qb))
            dinv = dpool.tile([64, 512], F32, tag="dinv", name=f"di{h}_{qb}")
            nc.vector.tensor_copy(draw_sb[:], po[64:65, :])
            nc.vector.reciprocal_approx_fast(drow_sb[:], draw_sb[:])
            nc.gpsimd.partition_broadcast(dinv[:], drow_sb[:])
            dst = aT0_sb if h < 2 else aT1_sb
            nc.vector.tensor_mul(
                dst[64 * (h % 2):64 * (h % 2) + 64, 512 * qb:512 * (qb + 1)],
                po[0:D, :], dinv[:])

        def oproj_m(qb, m):
            """One m-block of the out-proj partial for query block qb."""
            ps = gpool.tile([128, 512], F32, tag="g", name=f"ops{m}_{qb}")
            for j in range(2):
                aT = (aT0_sb, aT1_sb)[j]
                nc.tensor.matmul(ps[:], wo3[:, j, 128 * m:128 * (m + 1)],
                                 aT[:, 512 * qb:512 * (qb + 1)],
                                 start=(j == 0), stop=(j == 1))
            ev = ospool.tile([128, 512], BF16, tag="ev", name=f"oev{m}_{qb}")
            nc.vector.tensor_copy(ev[:], ps[:])
            nc.sync.dma_start(
                out=outT[128 * m:128 * (m + 1), 512 * qb:512 * (qb + 1)],
                in_=ev[:])

        # ---------------- emission ----------------
        # Pre-loop: Q^T(qb0) both planes, K^T plane 0.  First exp fires
        # once the first xq/xk halves land (~20us).
        for mb in range(2):
            qk_proj_part(wq3, xq3, qT_sb, mb, 0, 0, 1)
        for kb in range(NQB):
            qk_proj_part(wk3, xk3, kT_sb, 0, kb, 0, 1)

        # Slot schedule: one slot = one score group = one exp (~1.1us);
        # each slot carries <=~1.3us of extra PE work.
        #   qb0 h0: K^T plane-1 half-groups    (needed by h2 scores)
        #       h1: V' chunk 2p
        #       h2: V' chunk 2p+1 (pre), pv(h0) pair p
        #       h3: pv(h1) p, pv(h2) p-1, Q^T(qb1) quarters on p<4
        #       tail: pv(h2) pair 7        [pv(h3) rides in qb1's slots]
        #   qb>=1: pv(h-1) in-slot; fillers from the item list below;
        #       h3 additionally slot-lags pv(h3) by one pair, tail = pair 7.
        def run_qblock(qb, pre_items, post_items):
            lag = 2 if qb == 0 else 1
            for h in range(HC):
                for pair in range(8):
                    scores_pair(h, qb, pair)
                    for it in pre_items.get((h, pair), ()):
                        it()
                    if h >= lag:
                        pv_part(h - lag, qb, pair)
                    if h == HC - 1 and lag == 1 and pair >= 1:
                        pv_part(HC - 1, qb, pair - 1)
                    for it in post_items.get((h, pair), ()):
                        it()
            if qb == 0:
                pv_part(HC - 2, 0, 7)
            else:
                pv_part(HC - 1, qb, 7)

        pre0, post0 = {}, {}
        for p in range(8):
            post0[(0, p)] = [lambda kb=p // 2, part=p % 2:
                             qk_proj_part(wk3, xk3, kT_sb, 1, kb, part, 2)]
            post0[(1, p)] = [lambda tb=2 * p: v_proj_block(tb)]
            pre0[(2, p)] = [lambda tb=2 * p + 1: v_proj_block(tb)]
            post0[(3, p)] = [lambda pp=p - 1: pv_part(2, 0, pp)] if p >= 1 else []
            if p < 4:
                post0[(3, p)] = post0.get((3, p), []) + \
                    [lambda mb=p // 2, part=p % 2:
                     qk_proj_part(wq3, xq3, qT_sb, mb, 1, part, 2)]
        run_qblock(0, pre0, post0)
        xk_stack.close()
        xv_stack.close()

        for qb in range(1, NQB):
            items = []
            if qb == 1:
                items += [lambda p=p: pv_part(3, 0, p) for p in range(8)]
            opq = [lambda m=m, q=qb - 1: oproj_m(q, m) for m in range(8)]
            if qb < NQB - 1:
                qqs = [lambda mb=mb, part=part, q=qb + 1:
                       qk_proj_part(wq3, xq3, qT_sb, mb, q, part, 4)
                       for mb in range(2) for part in range(4)]
                inter = [x for pair in zip(opq, qqs) for x in pair]
            else:
                inter = opq
            items += inter
            sched = {}
            for s, it in enumerate(items):
                sched[(s // 8, s % 8)] = sched.get((s // 8, s % 8), []) + [it]
            run_qblock(qb, {}, sched)
            if qb == NQB - 2:
                xq_stack.close()
        pv_finish(HC - 1, NQB - 1) if (HC - 1, NQB - 1) in PV else None
        for m in range(8):
            oproj_m(NQB - 1, m)
        main.close()

    nc.compile()
    return nc


def _get_nc():
    if "nc" not in _CACHE:
        _CACHE["nc"] = build_nc()
    return _CACHE["nc"]


def _make_in_maps(q, k, v, Wq, Wk, Wv, Wo):
    bf = ml_dtypes.bfloat16
    q, k, v = np.asarray(q), np.asarray(k), np.asarray(v)
    qT = [np.ascontiguousarray(q[b].T).astype(bf) for b in range(B)]
    kT = [np.ascontiguousarray(k[b].T).astype(bf) for b in range(B)]
    vT = [np.ascontiguousarray(v[b].T).astype(bf) for b in range(B)]
    Wq, Wk, Wv, Wo = (np.asarray(x) for x in (Wq, Wk, Wv, Wo))
    wq_s = [np.ascontiguousarray(Wq[:, CB * g:CB * (g + 1)]).astype(bf)
            for g in range(4)]
    wk_s = [np.ascontiguousarray(Wk[:, CB * g:CB * (g + 1)]).astype(bf)
            for g in range(4)]
    wv_s = [np.ascontiguousarray(Wv[:, CB * g:CB * (g + 1)]).astype(bf)
            for g in range(4)]
    wo_s = [np.ascontiguousarray(Wo[CB * g:CB * (g + 1), :]).astype(bf)
            for g in range(4)]
    in_maps = []
    for c in range(8):
        b, g = c // 4, c % 4
        in_maps.append({
            "xqT": qT[b], "xkT": kT[b], "xvT": vT[b],
            "wq": wq_s[g], "wk": wk_s[g], "wv": wv_s[g], "wo": wo_s[g],
        })
    return in_maps


def _run(inputs, trace=False, **kw):
    nc = _get_nc()
    in_maps = _make_in_maps(inputs["q"], inputs["k"], inputs["v"],
                            inputs["Wq"], inputs["Wk"], inputs["Wv"], inputs["Wo"])
    res = None
    for attempt in range(3):
        try:
            res = run_bass_kernel_spmd(nc, in_maps, core_ids=list(range(8)),
                                       trace=trace, **kw)
            break
        except Exception:
            if attempt == 2:
                raise
            import time
            time.sleep(2.0)
    out = np.empty((B, N, C), np.float32)
    for b in range(B):
        acc = np.zeros((C, N), np.float32)
        for g in range(4):
            acc += res.results[4 * b + g]["outT"].astype(np.float32)
        out[b] = acc.T
    return out, res


def kernel(**inputs) -> np.ndarray:
    out, _ = _run(inputs, trace=False)
    return out
